# revision 1
# baseline (speedup 1.0000x reference)
"""Trainium2 Bass kernel for MultiHeadDoublyStochasticSelfAttention.

Problem: b=8, n=1024, f=768, h=12, d=64; 3-step Sinkhorn (eps=1, row/col/row)
on softmax-free exp scores, then attn @ v and output projection.

Sharding: one batch element per NeuronCore (8 cores). Weights replicated.

Math (per head), all in exp domain — no logsumexp needed:
  E = exp(S),  S = q' k^T  with the d^-0.5 scale folded into Wq on host.
  R_i = sum_j E_ij                  (fused into the exp pass via accum_out)
  nC_j = sum_i E_ij / R_i           (PE matvec with lhsT = 1/R)
  beta_j = 1 / nC_j
  Y'^T[d,i] = sum_j (beta_j v_jd) E_ij     (PE: lhsT = [beta*v | n*beta])
  row d=64 of Y'^T = n * sum_j E_ij beta_j  ->  gamma_i = 1/that
  out_head^T = gamma_i * Y'^T[:64]
Then out^T = Wo @ concat_heads(out_head^T) + bo, host transposes back.
"""

import sys

if "/opt/trn_rl_repo" not in sys.path:
    sys.path.insert(0, "/opt/trn_rl_repo")

from contextlib import ExitStack

import numpy as np

import concourse.bass as bass
import concourse.mybir as mybir
import concourse.tile as tile

B, N, F, H, D = 8, 1024, 768, 12, 64
PC = F // 128        # 6 f-chunks of 128
TC = N // 128        # 8 token chunks of 128
NH = 512             # fp32 moving-operand max
F32 = mybir.dt.float32
F32R = mybir.dt.float32r
EXP = mybir.ActivationFunctionType.Exp
IDENT = mybir.ActivationFunctionType.Identity


def _split_multi_waits(bir_bytes):
    """This container's walrus accepts at most ONE sync wait per instruction
    ("Too many sync wait commands"). Tile's semaphore pass attaches several.
    Rewrite the BIR: spill all but the last wait of each instruction onto
    same-engine NoOps placed directly before it (engines are in-order, so
    semantics are identical)."""
    import json

    d = json.loads(bir_bytes)
    uid = 0
    for fn in d["functions"]:
        for blk in fn["blocks"]:
            out = []
            for ins in blk["instructions"]:
                si = ins.get("sync_info")
                waits = (si or {}).get("on_wait") or []
                if len(waits) > 1:
                    for w in waits[:-1]:
                        uid += 1
                        out.append({
                            "debug": ins.get("debug", 0),
                            "engine": ins["engine"],
                            "ins": [], "outs": [],
                            "name": f"{ins['name']}-w{uid}",
                            "opcode": "NoOp",
                            "sync_info": {"on_update": [], "on_wait": [w]},
                            "text_hint": "split_wait",
                        })
                    si["on_wait"] = [waits[-1]]
                out.append(ins)
            blk["instructions"] = out
    return json.dumps(d).encode()


def build():
    nc = bass.Bass()
    xT = nc.declare_dram_parameter("xT", [F, N], F32R, isOutput=False)
    wqT = nc.declare_dram_parameter("wqT", [F, F], F32R, isOutput=False)
    wkT = nc.declare_dram_parameter("wkT", [F, F], F32R, isOutput=False)
    wvT = nc.declare_dram_parameter("wvT", [F, F], F32R, isOutput=False)
    woT = nc.declare_dram_parameter("woT", [F, F], F32R, isOutput=False)
    bo = nc.declare_dram_parameter("bo", [F], F32, isOutput=False)
    outT = nc.declare_dram_parameter("outT", [F, N], F32, isOutput=True)
    cscratch = nc.dram_tensor("cscratch", [H, N], F32)
    gscratch = nc.dram_tensor("gscratch", [H, N], F32)

    with tile.TileContext(nc) as tc, ExitStack() as ctx:
        perm = ctx.enter_context(tc.tile_pool(name="perm", bufs=1))
        qt = [perm.tile([128, N], F32R, name=f"qt{i}", tag=f"qt{i}") for i in range(PC)]
        kt = [perm.tile([128, N], F32R, name=f"kt{i}", tag=f"kt{i}") for i in range(PC)]
        # v augmented with a column of n (for the gamma row) per head
        vg = [perm.tile([128, H * (D + 1)], F32R, name=f"vg{i}", tag=f"vg{i}")
              for i in range(TC)]
        ofT = [perm.tile([128, N], F32R, name=f"ofT{i}", tag=f"ofT{i}")
               for i in range(PC)]
        wo_sb = [perm.tile([128, F], F32R, name=f"wo{i}", tag=f"wo{i}")
                 for i in range(PC)]
        bo_sb = perm.tile([128, PC], F32, name="bo_sb", tag="bo_sb")
        nc.sync.dma_start(out=bo_sb, in_=bo[:].rearrange("(c p) -> p c", p=128))
        for i in range(PC):
            nc.sync.dma_start(out=wo_sb[i], in_=woT[i * 128:(i + 1) * 128, :])
        for t in range(TC):
            # fill with n; the v-projection copies below overwrite the value
            # columns, leaving each head's 65th column = n (gamma-row trick)
            nc.vector.memset(vg[t].bitcast(F32), float(N))

        # ---------------- Phase A: q^T, k^T, v projections ----------------
        with tc.tile_pool(name="pxt", bufs=1) as pxt, \
             tc.tile_pool(name="pw", bufs=3 * PC) as pw, \
             tc.tile_pool(name="ppsa", bufs=4, space="PSUM") as ppsa:
            xt = [pxt.tile([128, N], F32R, name=f"xt{i}", tag=f"xt{i}")
                  for i in range(PC)]
            for i in range(PC):
                nc.sync.dma_start(out=xt[i], in_=xT[i * 128:(i + 1) * 128, :])

            for wdram, dst in ((wqT, qt), (wkT, kt)):
                w_sb = []
                for kc in range(PC):
                    w = pw.tile([128, F], F32R, name="w_sb", tag="w")
                    nc.sync.dma_start(out=w, in_=wdram[kc * 128:(kc + 1) * 128, :])
                    w_sb.append(w)
                for mc in range(PC):
                    for hf in range(2):
                        ps = ppsa.tile([128, NH], F32, name="ps_a", tag="psa")
                        for kc in range(PC):
                            nc.tensor.matmul(
                                ps,
                                (w_sb[kc][:, mc * 128:(mc + 1) * 128]),
                                (xt[kc][:, hf * NH:(hf + 1) * NH]),
                                start=(kc == 0), stop=(kc == PC - 1),
                            )
                        nc.vector.tensor_copy(dst[mc][:, hf * NH:(hf + 1) * NH], ps)

            wv_sb = []
            for kc in range(PC):
                w = pw.tile([128, F], F32R, name="wv_sb", tag="w")
                nc.sync.dma_start(out=w, in_=wvT[kc * 128:(kc + 1) * 128, :])
                wv_sb.append(w)
            for t in range(TC):
                for hf, fw in ((0, NH), (1, F - NH)):
                    ps = ppsa.tile([128, NH], F32, name="ps_v", tag="psa")
                    for kc in range(PC):
                        nc.tensor.matmul(
                            ps[:, :fw],
                            (xt[kc][:, t * 128:(t + 1) * 128]),
                            (wv_sb[kc][:, hf * NH:hf * NH + fw]),
                            start=(kc == 0), stop=(kc == PC - 1),
                        )
                    nhd = fw // D
                    src = ps[:, :fw].rearrange("p (h e) -> p h e", e=D)
                    dst3 = vg[t].rearrange("p (h e) -> p h e", e=D + 1)
                    nc.vector.tensor_copy(
                        dst3[:, hf * (NH // D):hf * (NH // D) + nhd, 0:D], src
                    )

        # ---------------- Phase B: per-head sinkhorn attention ----------------
        # Software-pipelined at head granularity: pass-2 (exp(ST), attn@v) of
        # head h-1 is interleaved chunk-by-chunk into pass-1 (exp(S), row sums,
        # weighted col sums) of head h, so TensorE and ScalarE both stay
        # continuously busy (keeps the PE HAM clock at 2.4 GHz).
        pe = ctx.enter_context(tc.tile_pool(name="pe", bufs=6))
        pet = ctx.enter_context(tc.tile_pool(name="pet", bufs=4))
        psml = ctx.enter_context(tc.tile_pool(name="psml", bufs=2))
        pps_s = ctx.enter_context(tc.tile_pool(name="pps_s", bufs=2, space="PSUM"))
        pps_cav = ctx.enter_context(tc.tile_pool(name="pps_cav", bufs=2, space="PSUM"))

        RG = 4          # chunks per reciprocal batch
        ST_LAG = 2      # pass-2 exp(ST) lags pass-1 chunks by this many iters
        AV_LAG = 4      # attn@v lags pass-1 chunks (covers the beta chain)

        def qk(h):
            hc, off = divmod(h, 2)
            off *= D
            return qt[hc][off:off + D, :], kt[hc][off:off + D, :]

        state = {}
        NITER = TC + AV_LAG
        for t in range(H + 1):
            h1 = t if t < H else None       # head in pass-1
            h2 = t - 1 if t >= 1 else None  # head in pass-2

            if h1 is not None:
                q1, k1 = qk(h1)
                c_ps = pps_cav.tile([D + 1, N], F32, name="c_ps", tag="pcav")
                e_tiles = [None] * TC
                raccs = []
                ris = []
            if h2 is not None:
                q2, k2 = qk(h2)
                av_ps = pps_cav.tile([D + 1, N], F32, name="av_ps", tag="pcav")
                binv2 = state.pop("binv")
                et_tiles = [None] * TC

            for it in range(NITER):
                # pass-1: scores + exp (fused row sums)
                ic = it
                if h1 is not None and ic < TC:
                    u = ic % RG
                    if u == 0:
                        racc = psml.tile([128, RG], F32, name="racc",
                                         tag="racc", bufs=3)
                        raccs.append(racc)
                    ps = pps_s.tile([128, N], F32, name="ps_s", tag="ps")
                    for jh in range(2):
                        nc.tensor.matmul(
                            ps[:, jh * NH:(jh + 1) * NH],
                            q1[:, ic * 128:(ic + 1) * 128],
                            k1[:, jh * NH:(jh + 1) * NH],
                            start=True, stop=True,
                        )
                    e_sb = pe.tile([128, N], F32R, name="e_sb", tag="E")
                    e_tiles[ic] = e_sb
                    nc.scalar.activation(e_sb, ps, EXP, accum_out=racc[:, u:u + 1])
                    if u == RG - 1:
                        ri = psml.tile([128, RG], F32R, name="ri", tag="ri", bufs=3)
                        with nc.allow_low_precision(reason="fp32r bits"):
                            nc.vector.reciprocal(ri, racc)
                        ris.append(ri)

                # pass-1: weighted col-sum matvecs, one reciprocal group late
                gi = (it - 1) // RG
                if h1 is not None and it >= 1 and (it - 1) % RG == RG - 1:
                    g0 = gi * RG
                    for u2 in range(RG):
                        for jh in range(2):
                            nc.tensor.matmul(
                                c_ps[0:1, jh * NH:(jh + 1) * NH],
                                ris[gi][:, u2:u2 + 1],
                                e_tiles[g0 + u2][:, jh * NH:(jh + 1) * NH],
                                start=(g0 + u2 == 0),
                                stop=(g0 + u2 == TC - 1),
                                skip_group_check=True,
                            )
                    if gi == TC // RG - 1:
                        # beta = 1/(n C), bounced through DRAM to col layout
                        crow = psml.tile([1, N], F32, name="crow", tag="crow",
                                         bufs=2)
                        nc.vector.tensor_copy(crow, c_ps[0:1, :])
                        nc.sync.dma_start(out=cscratch[h1:h1 + 1, :], in_=crow)
                        bcol = psml.tile([128, TC], F32, name="bcol", tag="bcol",
                                         bufs=2)
                        nc.sync.dma_start(
                            out=bcol,
                            in_=cscratch[h1:h1 + 1, :].rearrange(
                                "o (c p) -> (o p) c", p=128),
                        )
                        binv = psml.tile([128, TC], F32, name="binv", tag="binv",
                                         bufs=2)
                        nc.vector.reciprocal(binv, bcol)
                        state["binv"] = binv

                # pass-2: transposed scores + exp
                jc = it - ST_LAG
                if h2 is not None and 0 <= jc < TC:
                    ps2 = pps_s.tile([128, N], F32, name="ps_st", tag="ps")
                    for ih in range(2):
                        nc.tensor.matmul(
                            ps2[:, ih * NH:(ih + 1) * NH],
                            k2[:, jc * 128:(jc + 1) * 128],
                            q2[:, ih * NH:(ih + 1) * NH],
                            start=True, stop=True,
                        )
                    et2 = pet.tile([128, N], F32R, name="et2", tag="ET")
                    et_tiles[jc] = et2
                    nc.scalar.activation(et2, ps2, EXP)

                # pass-2: attn @ v
                jc = it - AV_LAG
                if h2 is not None and 0 <= jc < TC:
                    vs = psml.tile([128, D + 1], F32R, name="vs", tag="vs", bufs=3)
                    nc.vector.tensor_scalar_mul(
                        vs, vg[jc][:, h2 * (D + 1):(h2 + 1) * (D + 1)],
                        binv2[:, jc:jc + 1],
                    )
                    for ih in range(2):
                        nc.tensor.matmul(
                            av_ps[:, ih * NH:(ih + 1) * NH],
                            vs,
                            et_tiles[jc][:, ih * NH:(ih + 1) * NH],
                            start=(jc == 0), stop=(jc == TC - 1),
                            skip_group_check=True,
                        )

            if h2 is not None:
                # gamma = 1/(n T) from the extra ones-row, broadcast via DRAM
                grow = psml.tile([1, N], F32, name="grow", tag="crow", bufs=2)
                nc.vector.tensor_copy(grow, av_ps[D:D + 1, :])
                ginv = psml.tile([1, N], F32, name="ginv", tag="ginv", bufs=2)
                nc.vector.reciprocal(ginv, grow)
                nc.sync.dma_start(out=gscratch[h2:h2 + 1, :], in_=ginv)
                gb_sb = psml.tile([D, N], F32, name="gb_sb", tag="gb", bufs=2)
                gsrc = gscratch[h2:h2 + 1, :]
                nc.sync.dma_start(
                    out=gb_sb,
                    in_=bass.AP(tensor=gsrc.tensor, offset=gsrc.offset,
                                ap=[[0, D]] + list(gsrc.ap[1:])),
                )
                hcz, offz = divmod(h2, 2)
                offz *= D
                nc.vector.tensor_mul(
                    ofT[hcz][offz:offz + D, :], av_ps[0:D, :], gb_sb
                )

        # ---------------- Phase C: output projection + bias ----------------
        for mc in range(PC):
            ps = pps_s.tile([128, N], F32, name="ps_o", tag="ps")
            for hf in range(2):
                for kc in range(PC):
                    nc.tensor.matmul(
                        ps[:, hf * NH:(hf + 1) * NH],
                        (wo_sb[kc][:, mc * 128:(mc + 1) * 128]),
                        (ofT[kc][:, hf * NH:(hf + 1) * NH]),
                        start=(kc == 0), stop=(kc == PC - 1),
                    )
            o_sb = pe.tile([128, N], F32, name="o_sb", tag="E")
            nc.scalar.activation(o_sb, ps, IDENT, bias=bo_sb[:, mc:mc + 1])
            nc.sync.dma_start(out=outT[mc * 128:(mc + 1) * 128, :], in_=o_sb)

    orig_to_json = nc.to_json_bytes
    nc.to_json_bytes = lambda: _split_multi_waits(orig_to_json())
    return nc


_NC = None


def _get_nc():
    global _NC
    if _NC is None:
        _NC = build()
    return _NC


def make_in_maps(x, Wq, Wk, Wv, Wo, bo):
    scale = np.float32(D ** -0.5)
    wq_t = np.ascontiguousarray((Wq * scale).T.astype(np.float32))
    wk_t = np.ascontiguousarray(Wk.T.astype(np.float32))
    wv_t = np.ascontiguousarray(Wv.T.astype(np.float32))
    wo_t = np.ascontiguousarray(Wo.T.astype(np.float32))
    bo_c = np.ascontiguousarray(bo.astype(np.float32))
    maps = []
    for c in range(B):
        maps.append({
            "xT": np.ascontiguousarray(x[c].T.astype(np.float32)),
            "wqT": wq_t, "wkT": wk_t, "wvT": wv_t, "woT": wo_t, "bo": bo_c,
        })
    return maps


def kernel(x, Wq, Wk, Wv, Wo, bo):
    from concourse.bass_utils import run_bass_kernel_spmd

    x = np.asarray(x)
    nc = _get_nc()
    in_maps = make_in_maps(np.asarray(x), np.asarray(Wq), np.asarray(Wk),
                           np.asarray(Wv), np.asarray(Wo), np.asarray(bo))
    res = run_bass_kernel_spmd(nc, in_maps, core_ids=list(range(B)))
    out = np.stack([res.results[c]["outT"].T for c in range(B)], axis=0)
    return out.astype(np.float32)

